# revision 26
# baseline (speedup 1.0000x reference)
"""Trainium2 Bass kernel for nn_BlockAttentionResidual.

Transformer block: RMSNorm -> QKV -> RoPE -> block-diagonal causal attention
(4 blocks of 512) -> o-proj + residual -> RMSNorm -> SwiGLU FFN + residual.
Shapes: x [2, 2048, 2048], 32 heads x 64, inter 4096.

Sharding: 8 cores = (batch 2) x (4 sequence blocks of 512 tokens). The
attention mask is block-diagonal causal with block size 512, so each core's
512-token slice is fully independent -> no collectives.

On-device layout is "T layout" [feature, token] throughout. Matmuls run in
bf16 with fp32 PSUM accumulation.

v2 changes vs the first working kernel:
- rmsnorm1 is deferred: rstd[t] is a per-token scalar that commutes out of
  the feature contraction, so the q/k/v projections run directly on raw
  bf16 x (host-precast, DMA'd first) and rstd is folded into the runtime
  RoPE cos/sin tables (q/k) and a per-partition scale at the v copy. This
  removes the serial DMA->sumsq->norm->project chain at kernel start.
- attention is software-pipelined: scores of pair p issue before ctx of
  pair p-1, with the v-projection bursts interleaved, so the PE has
  independent work while the scalar engine runs the exps.
- softmax denominators use DVE reciprocal + a K=1 broadcast matmul instead
  of per-head scalar Ln/Exp ACTIVATEs (each cost (N+352)/1.2 ns and used 1
  of 128 lanes).
- exp chunks packed 4->3 ACTIVATEs per head (512 | 384+128 | 256 cols).
- v1 copies moved from scalar to gpsimd; squares for sumsq moved from
  scalar to DVE.
"""

import math
from contextlib import ExitStack

import ml_dtypes
import numpy as np

import concourse.bass as bass
import concourse.mybir as mybir
import concourse.tile as tile
from concourse.bass_utils import run_bass_kernel_spmd
from concourse.vector_clock import ScopedClock

F32 = mybir.dt.float32
BF16 = mybir.dt.bfloat16
NPBF16 = ml_dtypes.bfloat16

EPS = 1e-5
ROPE_THETA = 10000.0


# --- workaround: this walrus build allows only one sem wait per CTRL-queue
# instruction (Drain/NoOp), but Tile's tail drain aggregates every
# outstanding wait onto a single SP Drain. Spread them over SP NOPs.
def _patched_drain_and_barrier(self, tick_clock, wait_clock):
    nop_inst = self.nc.sync.nop(nofuse=True)
    wait_clock.add_sem_waits(
        nop_inst.ins, ScopedClock({None: tick_clock.global_clock})
    )
    si = nop_inst.ins.sync_info
    waits = list(si.on_wait) if si is not None else []
    if len(waits) > 1:
        si.on_wait = waits[:1]
        for w in waits[1:]:
            n2 = self.nc.sync.nop(nofuse=True)
            if n2.ins.sync_info is None:
                n2.ins.sync_info = mybir.SyncInfo(on_wait=[w], on_update=[])
            else:
                n2.ins.sync_info.on_wait = [w]
    self.nc.sync.drain()
    self.nc.all_engine_barrier()
    assert self.sems is not None
    popped = self.nc._tile_sem_poison_stack.pop()
    assert popped is self._sem_poison
    self.nc.clear_and_free_semaphores(list(self.sems.allocated().values()))
    self.nc.all_engine_barrier()


tile.TileContext._drain_and_barrier = _patched_drain_and_barrier


def _split_excess_waits(nc, maxw=1):
    """This walrus build rejects instructions carrying more than one sync
    wait; hoist extras onto single-wait NOPs queued just before on the same
    engine."""
    fn = nc.m.functions[0]
    for bb in fn.blocks:
        out = []
        changed = False
        for inst in bb.instructions:
            si = getattr(inst, "sync_info", None)
            waits = list(si.on_wait) if si is not None else []
            if len(waits) > maxw:
                changed = True
                for w in waits[:-maxw]:
                    nop = mybir.InstNoOp(
                        name=nc.get_next_instruction_name(), ins=[], outs=[])
                    nop.engine = inst.engine
                    nop.sync_info = mybir.SyncInfo(on_wait=[w], on_update=[])
                    out.append(nop)
                si.on_wait = waits[-maxw:]
            out.append(inst)
        if changed:
            bb.instructions = out


class Cfg:
    def __init__(self, T=512, C=2048, H=32, D=64, I=4096):
        self.T = T          # tokens per core (one attention block)
        self.C = C          # hidden
        self.H = H          # heads
        self.D = D          # head dim (must be 64)
        self.I = I          # ffn inner
        assert D == 64 and C == H * D
        assert T % 128 == 0 and C % 128 == 0 and I % 128 == 0


def build_program(cfg: Cfg):
    T, C, H, D, I = cfg.T, cfg.C, cfg.H, cfg.D, cfg.I
    CT = C // 128            # hidden tiles
    KT = T // 128            # token chunks (and attention k-chunks)
    QKN = 2 * C              # q+k feature rows
    ICH = I // 128           # ffn inner chunks
    scale = 1.0 / math.sqrt(D)
    OG = 8                   # psum group width for dense matmul phases

    nc = bass.Bass("TRN2", target_bir_lowering=False, debug=False)

    xbT = nc.dram_tensor("xbT", (C, T), BF16, kind="ExternalInput").ap()
    xT = nc.dram_tensor("xT", (C, T), F32, kind="ExternalInput").ap()
    w_qkT = nc.dram_tensor("w_qkT", (C, QKN), BF16, kind="ExternalInput").ap()
    w_vT = nc.dram_tensor("w_vT", (C, C), BF16, kind="ExternalInput").ap()
    w_oT = nc.dram_tensor("w_oT", (C, C), BF16, kind="ExternalInput").ap()
    w_upT = nc.dram_tensor("w_upT", (C, 2 * I), BF16, kind="ExternalInput").ap()
    w_downT = nc.dram_tensor("w_downT", (I, C), BF16, kind="ExternalInput").ap()
    cosT2 = nc.dram_tensor("cosT2", (128, T), F32, kind="ExternalInput").ap()
    nsinT2 = nc.dram_tensor("nsinT2", (128, T), F32, kind="ExternalInput").ap()
    trimask = nc.dram_tensor("trimask", (128, 128), BF16, kind="ExternalInput").ap()
    pswap = nc.dram_tensor("pswap", (128, 128), BF16, kind="ExternalInput").ap()
    outT = nc.dram_tensor("outT", (C, T), F32, kind="ExternalOutput").ap()

    with tile.TileContext(nc) as tc, ExitStack() as ctx:
        consts = ctx.enter_context(tc.tile_pool(name="consts", bufs=1))
        # xb slots also serve h2 (xb dies after the v projection)
        xb_pool = ctx.enter_context(tc.tile_pool(name="xb", bufs=CT))
        # xt slots also serve x2 (x dies at the o-proj residual add); +2
        # rolling slots so the x2 alloc never waits on its own free.
        xt_pool = ctx.enter_context(tc.tile_pool(name="xt", bufs=CT + 2))
        qk_pool = ctx.enter_context(
            tc.tile_pool(name="qk", bufs=max(2 * CT, ICH)))
        v_pool = ctx.enter_context(tc.tile_pool(name="v", bufs=KT))
        ctx_pool = ctx.enter_context(tc.tile_pool(name="ctx", bufs=CT))
        wsl_pool = ctx.enter_context(tc.tile_pool(name="wsl", bufs=18))
        tr_pool = ctx.enter_context(tc.tile_pool(name="tr", bufs=4))
        e_pool = ctx.enter_context(tc.tile_pool(name="e", bufs=12))
        sm_pool = ctx.enter_context(tc.tile_pool(name="sm", bufs=1))
        rec_pool = ctx.enter_context(tc.tile_pool(name="rec", bufs=3))
        rsrc_pool = ctx.enter_context(tc.tile_pool(name="rsrc", bufs=6))
        ps_pool = ctx.enter_context(
            tc.tile_pool(name="ps", bufs=8, space="PSUM"))

        _nm = [0]

        def named(base):
            _nm[0] += 1
            return f"{base}{_nm[0]}"

        def ps_tile():
            return ps_pool.tile([128, T], F32, tag="ps", name=named("ps"))

        # ---- load xbT first: the qk matmuls depend only on it, so it must
        # lead the issue queues. The fp32 xT (residual stream) is deferred
        # until after the qk phase - it's only consumed at the o-proj
        # residual add, and loading it early would steal HBM bandwidth from
        # the qk weight stream.
        # cos/nsin fp32 staging rides wsl slots (2KB each, recycled by the
        # weight stream after the rope tables are folded)
        cosf = wsl_pool.tile([128, T], F32, tag="wsl", name=named("t"))
        nc.sync.dma_start(cosf[:], cosT2[:])
        nsinf = wsl_pool.tile([128, T], F32, tag="wsl", name=named("t"))
        nc.scalar.dma_start(nsinf[:], nsinT2[:])
        xb = []
        for ci in range(CT):
            t = xb_pool.tile([128, T], BF16, tag="xb", name=named("t"))
            eng = nc.sync if ci % 2 == 0 else nc.scalar
            eng.dma_start(t[:], xbT[ci * 128:(ci + 1) * 128, :])
            xb.append(t)

        # ---- constants
        sb_tri = consts.tile([128, 128], BF16)
        nc.sync.dma_start(sb_tri[:], trimask[:])
        sb_psw = consts.tile([128, 128], BF16)
        nc.scalar.dma_start(sb_psw[:], pswap[:])
        ones_col = consts.tile([128, 1], BF16)
        nc.vector.memset(ones_col[:], 1.0)
        ones128 = consts.tile([128, 128], BF16)
        nc.gpsimd.memset(ones128[:], 1.0)
        one1 = consts.tile([1, 1], BF16)
        nc.gpsimd.memset(one1[:], 1.0)
        eps_t = consts.tile([1, 1], F32)
        nc.vector.memset(eps_t[:], EPS)
        cos_r = consts.tile([128, T], BF16)   # cos * rstd (runtime)
        nsin_r = consts.tile([128, T], BF16)  # -sin(perm) * rstd (runtime)
        rstdc = consts.tile([128, KT], F32)   # rstd as per-token-chunk cols

        # ---- q/k projection (T layout, raw x) + RoPE (rstd-folded tables)
        # The sumsq chain for rmsnorm1's rstd is computed from the bf16 x
        # (error on rstd ~1e-4, irrelevant) and interleaved into the first
        # og group so the PE never waits on a norm before projecting.
        ps_ss = ps_tile()
        sm_rstd = [None]  # rstd row tile, set inside the og loop

        def emit_rstd_tail():
            # Ln/Exp for rstd (scalar), broadcast matmul, rope-table folds,
            # and the per-token-chunk rstd columns for the v copy.
            s_sb = sm_pool.tile([1, T], F32, tag="s1", name=named("t"))
            nc.scalar.activation(
                s_sb[:], ps_ss[0:1, :], mybir.ActivationFunctionType.Ln,
                bias=eps_t[:], scale=1.0 / C,
            )
            rstd = sm_pool.tile([1, T], BF16, tag="s2", name=named("t"))
            nc.scalar.activation(
                rstd[:], s_sb[:], mybir.ActivationFunctionType.Exp,
                scale=-0.5,
            )
            sm_rstd[0] = rstd

        def emit_rstd_bcast():
            rstd = sm_rstd[0]
            ps_bcr = ps_tile()
            nc.tensor.matmul(ps_bcr[:, :], ones128[0:1, :], rstd[:],
                             start=True, stop=True)
            nc.vector.tensor_mul(cos_r[:], cosf[:], ps_bcr[:, :])
            nc.vector.tensor_mul(nsin_r[:], nsinf[:], ps_bcr[:, :])
            # rstd as per-token-chunk [128,1] columns (for the v copy
            # scale): KT tiny K=1 matmuls transpose the row into columns.
            ps_rc = ps_tile()
            for tc_i in range(KT):
                nc.tensor.matmul(
                    ps_rc[:, tc_i:tc_i + 1],
                    rstd[0:1, tc_i * 128:(tc_i + 1) * 128],
                    one1[:],
                    start=(tc_i == 0), stop=(tc_i == KT - 1),
                )
            nc.vector.tensor_copy(rstdc[:, :], ps_rc[:, 0:KT])
        qkrot = [None] * (QKN // 128)
        n_och = QKN // 128
        rope_pend = []

        def flush_rope(n=99):
            # dripped between matmul bursts so the swap matmul's inputs are
            # ready and the DVE work is spread
            for _ in range(min(n, len(rope_pend))):
                idx, src = rope_pend.pop(0)
                a = tr_pool.tile([128, T], BF16, tag="trb2", name=named("t"))
                nc.vector.tensor_mul(a[:], src[:], cos_r[:])
                m = tr_pool.tile([128, T], BF16, tag="trb3", name=named("t"))
                nc.vector.tensor_mul(m[:], src[:], nsin_r[:])
                ps_b = ps_tile()
                nc.tensor.matmul(ps_b[:, :], sb_psw[:], m[:],
                                 start=True, stop=True)
                rot = qk_pool.tile([128, T], BF16, tag="qk", name=named("t"))
                nc.vector.tensor_add(rot[:], a[:], ps_b[:, :])
                qkrot[idx] = rot

        # og-PAIR structure: one 256KB DMA per ci covers 8 out-tiles; pass A
        # computes tiles [0:4] while the DMA stream runs, pass B computes
        # tiles [4:8] DMA-free (the queues run ahead prefetching the next
        # pair). 256KB transfers double the per-queue DMA efficiency vs the
        # 128KB slices this replaced.
        for ogp in range(n_och // 8):
            wts = []
            for pas in range(2):
                pss = [ps_tile() for _ in range(4)]
                for ci in range(CT):
                    if pas == 0:
                        wt8 = wsl_pool.tile([128, OG * 128], BF16, tag="wsl",
                                            name=named("t"))
                        # opposite parity to the xb stream so ci=0's weight
                        # slice isn't queued behind all 16 xb tiles
                        eng = nc.scalar if ci % 2 == 0 else nc.sync
                        eng.dma_start(
                            wt8[:],
                            w_qkT[ci * 128:(ci + 1) * 128,
                                  ogp * 1024:(ogp + 1) * 1024],
                        )
                        wts.append(wt8)
                    wt8 = wts[ci]
                    for j in range(4):
                        nc.tensor.matmul(
                            pss[j][:, :],
                            wt8[:, (pas * 4 + j) * 128:(pas * 4 + j + 1) * 128],
                            xb[ci][:],
                            start=(ci == 0), stop=(ci == CT - 1),
                        )
                    if ogp == 0 and pas == 0:
                        # sumsq chain rides along with the first pass
                        sq = tr_pool.tile([128, T], BF16, tag="trb0",
                                          name=named("t"))
                        nc.vector.tensor_mul(sq[:], xb[ci][:], xb[ci][:])
                        nc.tensor.matmul(
                            ps_ss[0:1, :], ones_col[:], sq[:],
                            start=(ci == 0), stop=(ci == CT - 1),
                        )
                    elif ogp == 0 and pas == 1 and ci == 1:
                        emit_rstd_bcast()
                    if ci % 2 == 1:
                        flush_rope(1)
                if ogp == 0 and pas == 0:
                    emit_rstd_tail()
                flush_rope(99 if ogp == 0 and pas == 0 else 0)
                for j in range(4):
                    src = rsrc_pool.tile([128, T], BF16, tag="ropesrc",
                                         name=named("t"))
                    nc.scalar.copy(src[:], pss[j][:, :])
                    rope_pend.append((ogp * 8 + pas * 4 + j, src))
        flush_rope()

        # ---- v1 tiles: per k-chunk [128 tok, H*(D+1)] with a ones column
        # appended per head so the ctx matmul also produces the softmax
        # denominator row for free.
        v1 = [v_pool.tile([128, H * 65], BF16, tag="v1", name=named("t")) for _ in range(KT)]
        for tc_i in range(KT):
            ones_slots = v1[tc_i].rearrange("p (h e) -> p h e", e=65)[:, :, 64]
            nc.gpsimd.memset(ones_slots, 1.0)

        ctxT = [ctx_pool.tile([128, T], BF16, tag="ctx", name=named("t")) for _ in range(CT)]

        def v_super(s):
            # token-stationary v projection for feature chunk [s*1024,
            # (s+1)*1024) = heads 16s..16s+15. One 256KB DMA per ci, two
            # compute passes of 512 features each; rstd applied
            # per-partition (token) at the copy into v1.
            wts = []
            for half in range(2):
                pss = [ps_tile() for _ in range(KT)]
                for ci in range(CT):
                    if half == 0:
                        wt8 = wsl_pool.tile([128, OG * 128], BF16, tag="wsl",
                                            name=named("t"))
                        eng = nc.sync if ci % 2 == 0 else nc.scalar
                        eng.dma_start(
                            wt8[:],
                            w_vT[ci * 128:(ci + 1) * 128,
                                 s * 1024:(s + 1) * 1024],
                        )
                        wts.append(wt8)
                    wt8 = wts[ci]
                    for tc_i in range(KT):
                        nc.tensor.matmul(
                            pss[tc_i][:, :],
                            xb[ci][:, tc_i * 128:(tc_i + 1) * 128],
                            wt8[:, half * 512:(half + 1) * 512],
                            start=(ci == 0), stop=(ci == CT - 1),
                        )
                h0 = s * 16 + half * 8
                for tc_i in range(KT):
                    dst = v1[tc_i][:, h0 * 65:(h0 + 8) * 65].rearrange(
                        "p (h e) -> p h e", e=65)[:, :, 0:64]
                    srcap = pss[tc_i][:, :].rearrange("p (h e) -> p h e", e=64)
                    nc.vector.tensor_scalar_mul(
                        dst, srcap, rstdc[:, tc_i:tc_i + 1])

        # scores / exp for one head: 4 score matmuls packed into psum
        # banks -> exp ACTIVATEs; the kt2 chunk (256 cols) of the two heads
        # of a pair shares one bank/ACTIVATE (s3 passed in for the second
        # head). Returns es slices for the ctx matmuls.
        def attn_scores(h, s3=None, e3=None):
            q_t = qkrot[(h * D) // 128]
            k_t = qkrot[(C + h * D) // 128]
            ro = (h * D) % 128
            s1 = ps_tile()   # kt0: [0:512]
            s2 = ps_tile()   # kt1: [0:384], kt3: [384:512]
            first = s3 is None
            s3 = ps_tile()   # kt2: [0:256]
            o3 = 0
            place = {0: (s1, 0), 1: (s2, 0), 3: (s2, 384), 2: (s3, o3)}
            for kt in (0, 1, 3, 2):
                ncols = T - kt * 128
                ps, co = place[kt]
                nc.tensor.matmul(
                    ps[:, co:co + ncols],
                    k_t[ro:ro + D, kt * 128:(kt + 1) * 128],
                    q_t[ro:ro + D, kt * 128:],
                    start=True, stop=True,
                )
            e1 = e_pool.tile([128, T], BF16, tag="e", name=named("t"))
            nc.scalar.activation(
                e1[:, :], s1[:, :],
                mybir.ActivationFunctionType.Exp, scale=scale,
            )
            e2 = e_pool.tile([128, T], BF16, tag="e", name=named("t"))
            nc.scalar.activation(
                e2[:, :], s2[:, :],
                mybir.ActivationFunctionType.Exp, scale=scale,
            )
            e3 = e_pool.tile([128, T], BF16, tag="e", name=named("t"))
            nc.scalar.activation(
                e3[:, :256], s3[:, :256],
                mybir.ActivationFunctionType.Exp, scale=scale,
            )
            # causal mask on the diagonal 128x128 of each chunk (gpsimd:
            # operands are all SBUF and DVE/scalar are busier here)
            nc.gpsimd.tensor_mul(e1[:, 0:128], e1[:, 0:128], sb_tri[:])
            nc.gpsimd.tensor_mul(e2[:, 0:128], e2[:, 0:128], sb_tri[:])
            nc.gpsimd.tensor_mul(e2[:, 384:512], e2[:, 384:512], sb_tri[:])
            nc.gpsimd.tensor_mul(e3[:, 0:128], e3[:, 0:128], sb_tri[:])
            es = {0: e1[:, 0:512], 1: e2[:, 0:384], 2: e3[:, 0:256],
                  3: e2[:, 384:512]}
            return es, s3, e3

        def attn_ctx(h, es):
            # unnormalized context + denominator row
            ctx_ps = ps_tile()
            for kt in range(KT):
                nc.tensor.matmul(
                    ctx_ps[0:65, kt * 128:],
                    v1[kt][:, h * 65:(h + 1) * 65],
                    es[kt],
                    start=(kt == 0), stop=(kt == KT - 1),
                    skip_group_check=True,
                )
            return ctx_ps

        def ctx_fin(p, es_a, es_b):
            cps_a = attn_ctx(2 * p, es_a)
            cps_b = attn_ctx(2 * p + 1, es_b)
            # raw context rows straight to their SBUF home; denominators
            # Ln'd straight from psum (base-64 read, base-0 write: legal);
            # psums free after one cast + one Ln each.
            nc.vector.tensor_copy(ctxT[p][0:64, :], cps_a[0:64, :])
            nc.vector.tensor_copy(ctxT[p][64:128, :], cps_b[0:64, :])
            rec_b = rec_pool.tile([128, T], BF16, tag="recb", name=named("t"))
            dlt = sm_pool.tile([128, T], F32, tag="dl", bufs=2, name=named("t"))
            nc.scalar.activation(
                dlt[0:1, :], cps_a[64:65, :], mybir.ActivationFunctionType.Ln,
            )
            nc.scalar.activation(
                dlt[64:65, :], cps_b[64:65, :], mybir.ActivationFunctionType.Ln,
            )
            nc.scalar.activation(
                rec_b[0:1, :], dlt[0:1, :],
                mybir.ActivationFunctionType.Exp, scale=-1.0,
            )
            nc.scalar.activation(
                rec_b[64:65, :], dlt[64:65, :],
                mybir.ActivationFunctionType.Exp, scale=-1.0,
            )
            ps_bc = ps_tile()
            nc.tensor.matmul(ps_bc[0:64, :], ones128[0:1, 0:64],
                             rec_b[0:1, :], start=True, stop=True)
            nc.tensor.matmul(ps_bc[64:128, :], ones128[64:65, 0:64],
                             rec_b[64:65, :], start=True, stop=True)
            nc.vector.tensor_mul(ctxT[p][:, :], ctxT[p][:, :], ps_bc[:, :])

        # ---- attention: pair-depth-1 pipeline (scores of pair p issue
        # before ctx+finalize of pair p-1), v bursts every 8 pairs, fp32
        # residual stream trickled in between.
        NP = H // 2
        xt = []
        prev = None
        for p in range(NP):
            if p % (NP // 2) == 0:
                v_super(p // (NP // 2))
            if p < CT:
                t = xt_pool.tile([128, T], F32, tag="xt", name=named("t"))
                eng = nc.sync if p % 2 == 0 else nc.scalar
                eng.dma_start(t[:], xT[p * 128:(p + 1) * 128, :])
                xt.append(t)
            es_a, _, _ = attn_scores(2 * p)
            es_b, _, _ = attn_scores(2 * p + 1)
            if prev is not None:
                ctx_fin(*prev)
            prev = (p, es_a, es_b)
        ctx_fin(*prev)

        # ---- o-proj (T layout) + residual -> x2T; og-pair two-pass like
        # qk, sumsq2 interleaved.
        x2t = [None] * CT
        ps_ss2 = ps_tile()
        for ogp in range(CT // 8):
            wts = []
            for pas in range(2):
                pss = [ps_tile() for _ in range(4)]
                for ci in range(CT):
                    if pas == 0:
                        wt8 = wsl_pool.tile([128, OG * 128], BF16, tag="wsl",
                                            name=named("t"))
                        eng = nc.sync if ci % 2 == 0 else nc.scalar
                        eng.dma_start(
                            wt8[:],
                            w_oT[ci * 128:(ci + 1) * 128,
                                 ogp * 1024:(ogp + 1) * 1024],
                        )
                        wts.append(wt8)
                    wt8 = wts[ci]
                    for j in range(4):
                        nc.tensor.matmul(
                            pss[j][:, :],
                            wt8[:, (pas * 4 + j) * 128:(pas * 4 + j + 1) * 128],
                            ctxT[ci][:],
                            start=(ci == 0), stop=(ci == CT - 1),
                        )
                for j in range(4):
                    oi = ogp * 8 + pas * 4 + j
                    x2 = xt_pool.tile([128, T], F32, tag="xt", name=named("t"))
                    nc.vector.tensor_add(x2[:], xt[oi][:], pss[j][:, :])
                    x2t[oi] = x2
                    sq2 = tr_pool.tile([128, T], BF16, tag="trb0", name=named("t"))
                    nc.vector.tensor_mul(sq2[:], x2[:], x2[:])
                    nc.tensor.matmul(
                        ps_ss2[0:1, :], ones_col[:], sq2[:],
                        start=(oi == 0), stop=(oi == CT - 1),
                    )

        # ---- rmsnorm 2 (explicit: the up GEMM's gate half feeds a
        # nonlinearity, so rstd can't be deferred through it)
        s_sb2 = sm_pool.tile([1, T], F32, tag="s1", name=named("t"))
        nc.scalar.activation(
            s_sb2[:], ps_ss2[0:1, :], mybir.ActivationFunctionType.Ln,
            bias=eps_t[:], scale=1.0 / C,
        )
        rstd2 = sm_pool.tile([1, T], BF16, tag="s2", name=named("t"))
        nc.scalar.activation(
            rstd2[:], s_sb2[:], mybir.ActivationFunctionType.Exp,
            scale=-0.5,
        )
        ps_bc2 = ps_tile()
        nc.tensor.matmul(ps_bc2[:, :], ones128[0:1, :], rstd2[:],
                         start=True, stop=True)
        h2t = []
        for ci in range(CT):
            h = xb_pool.tile([128, T], BF16, tag="xb", name=named("t"))
            nc.vector.tensor_mul(h[:], x2t[ci][:], ps_bc2[:, :])
            h2t.append(h)

        # ---- FFN up + swiglu -> actT (bf16, I rows)
        actT = [None] * ICH
        GG = min(4, ICH)  # gate chunks per group (paired with value chunks)
        for gg in range(0, ICH, GG):
            g = min(GG, ICH - gg)
            ps_gate = [ps_tile() for _ in range(g)]
            ps_val = [ps_tile() for _ in range(g)]
            for ci in range(CT):
                wt = wsl_pool.tile([128, OG * 128], BF16, tag="wsl", name=named("t"))
                nc.sync.dma_start(
                    wt[:, :g * 128],
                    w_upT[ci * 128:(ci + 1) * 128, gg * 128:(gg + g) * 128],
                )
                nc.scalar.dma_start(
                    wt[:, GG * 128:(GG + g) * 128],
                    w_upT[ci * 128:(ci + 1) * 128,
                          I + gg * 128:I + (gg + g) * 128],
                )
                for j in range(g):
                    nc.tensor.matmul(
                        ps_gate[j][:, :], wt[:, j * 128:(j + 1) * 128],
                        h2t[ci][:],
                        start=(ci == 0), stop=(ci == CT - 1),
                    )
                    nc.tensor.matmul(
                        ps_val[j][:, :],
                        wt[:, (GG + j) * 128:(GG + j + 1) * 128],
                        h2t[ci][:],
                        start=(ci == 0), stop=(ci == CT - 1),
                    )
            for j in range(g):
                sg = tr_pool.tile([128, T], BF16, tag="trb1", name=named("t"))
                nc.scalar.activation(
                    sg[:], ps_gate[j][:, :],
                    mybir.ActivationFunctionType.Silu,
                )
                a = qk_pool.tile([128, T], BF16, tag="qk", name=named("t"))
                nc.vector.tensor_mul(a[:], sg[:], ps_val[j][:, :])
                actT[gg + j] = a

        # ---- FFN down + residual -> outT
        OGD = 4
        for og in range(0, CT, OGD):
            g = min(OGD, CT - og)
            pss = [ps_tile() for _ in range(g)]
            for ii in range(ICH):
                wt = wsl_pool.tile([128, OG * 128], BF16, tag="wsl", name=named("t"))
                eng = nc.sync if ii % 2 == 0 else nc.scalar
                eng.dma_start(
                    wt[:, :g * 128],
                    w_downT[ii * 128:(ii + 1) * 128, og * 128:(og + g) * 128],
                )
                for j in range(g):
                    nc.tensor.matmul(
                        pss[j][:, :],
                        wt[:, j * 128:(j + 1) * 128],
                        actT[ii][:],
                        start=(ii == 0), stop=(ii == ICH - 1),
                    )
            for j in range(g):
                o_sb = tr_pool.tile([128, T], F32, tag="trf", name=named("t"))
                nc.vector.tensor_add(o_sb[:], x2t[og + j][:], pss[j][:, :])
                eng2 = nc.sync if j % 2 == 0 else nc.scalar
                eng2.dma_start(
                    outT[(og + j) * 128:(og + j + 1) * 128, :], o_sb[:],
                )

    _split_excess_waits(nc)
    return nc


def make_core_inputs(cfg: Cfg, x_shard, w_qkv, w_o, w_up, w_down,
                     attn_norm_w, ffn_norm_w, pos0, shared):
    """Host-side prep of one core's input map. x_shard [T, C] fp32.
    `shared` caches the (identical) weight arrays across cores."""
    T, C, D = cfg.T, cfg.C, cfg.D
    if not shared:
        nw1 = attn_norm_w.astype(np.float32)[:, None]   # [C, 1]
        nw2 = ffn_norm_w.astype(np.float32)[:, None]
        shared["w_qkT"] = np.ascontiguousarray(
            w_qkv[:2 * C].T * nw1).astype(NPBF16)
        shared["w_vT"] = np.ascontiguousarray(
            w_qkv[2 * C:3 * C].T * nw1).astype(NPBF16)
        shared["w_oT"] = np.ascontiguousarray(w_o.T).astype(NPBF16)
        shared["w_upT"] = np.ascontiguousarray(w_up.T * nw2).astype(NPBF16)
        shared["w_downT"] = np.ascontiguousarray(w_down.T).astype(NPBF16)
        k_idx = np.arange(128)
        shared["trimask"] = (
            k_idx[:, None] <= k_idx[None, :]).astype(NPBF16)
        psw = np.zeros((128, 128), dtype=NPBF16)
        psw[k_idx ^ 32, k_idx] = 1.0  # lhsT[j, p] = 1 iff j == p ^ 32
        shared["pswap"] = psw
    inv = (1.0 / ROPE_THETA ** (np.arange(0, D, 2) / D)).astype(np.float64)
    pos = np.arange(pos0, pos0 + T, dtype=np.float64)
    fr = np.outer(pos, inv)                       # [T, D/2]
    emb = np.concatenate([fr, fr], axis=-1)       # [T, D]
    cosT = np.cos(emb).T.astype(np.float32)       # [D, T]
    sinT = np.sin(emb).T.astype(np.float32)
    nsinT = sinT.copy()
    nsinT[:D // 2] *= -1.0
    reps = 128 // D
    nsin2 = np.tile(nsinT, (reps, 1))
    perm = np.arange(128) ^ 32
    s2 = nsin2[perm]          # s2[p] = nsin2[p ^ 32]
    xc = np.ascontiguousarray(x_shard.T)
    return {
        "xT": xc.astype(np.float32),
        "xbT": xc.astype(NPBF16),
        "cosT2": np.tile(cosT, (reps, 1)).astype(np.float32),
        "nsinT2": s2.astype(np.float32),
        **shared,
    }


def kernel(x, attn_norm_w, ffn_norm_w, w_qkv, w_o, w_up, w_down,
           _trace=False, _tmpdir=None):
    x = np.asarray(x, dtype=np.float32)
    attn_norm_w = np.asarray(attn_norm_w, dtype=np.float32)
    ffn_norm_w = np.asarray(ffn_norm_w, dtype=np.float32)
    w_qkv = np.asarray(w_qkv, dtype=np.float32)
    w_o = np.asarray(w_o, dtype=np.float32)
    w_up = np.asarray(w_up, dtype=np.float32)
    w_down = np.asarray(w_down, dtype=np.float32)

    B, S, C = x.shape
    cfg = Cfg(T=512, C=C, H=C // 64, D=64, I=2 * C)
    n_blocks = S // cfg.T
    assert B * n_blocks == 8

    nc = build_program(cfg)

    shared = {}
    in_maps = []
    for core in range(8):
        b, blk = divmod(core, n_blocks)
        sl = slice(blk * cfg.T, (blk + 1) * cfg.T)
        in_maps.append(make_core_inputs(
            cfg, x[b, sl], w_qkv, w_o, w_up, w_down,
            attn_norm_w, ffn_norm_w, pos0=blk * cfg.T, shared=shared,
        ))

    res = run_bass_kernel_spmd(
        nc, in_maps, core_ids=list(range(8)),
        trace=_trace, tmpdir=_tmpdir,
    )

    out = np.empty((B, S, C), dtype=np.float32)
    for core in range(8):
        b, blk = divmod(core, n_blocks)
        sl = slice(blk * cfg.T, (blk + 1) * cfg.T)
        out[b, sl] = res.results[core]["outT"].T
    kernel.last_result = res
    return out
